# revision 1
# baseline (speedup 1.0000x reference)
"""Trainium2 Bass kernel for nn_Discriminator (GRU-like recurrent discriminator).

Math (per batch row):
    belta = exp(-relu(td @ Wb^T + bb))                       # (T, H)
    for t in 0..T-1:
        s = belta[t] * s
        u = sigmoid(s @ W1h^T + x[t] @ W1x^T + b1)
        r = sigmoid(s @ W2h^T + x[t] @ W2x^T + b2)
        n = tanh((r*s) @ W3h^T + x[t] @ W3x^T + b3)
        s = (1-u)*s + u*n
    out = sigmoid(s @ Wo^T + bo)

Strategy: data-parallel over 8 cores on the batch dim (B=256 -> 32/core).
Phase 1 precomputes belta and the x-contributions of all gates as dense
matmuls (PE-efficient), stored t-major in DRAM.  Phase 2 runs the T=96
sequential steps with the state held *transposed* in SBUF
([h=128 partitions x 8 chunks, b=32]) so no per-step transposes are needed;
gate matmuls are weight-stationary bf16 (fp32 PSUM accumulation).
"""

import numpy as np
import ml_dtypes

B, T, IN, H = 256, 96, 512, 1024
NCORES = 8
BS = B // NCORES      # 32 batch rows per core
HC = H // 128         # 8 hidden chunks
KC = IN // 128        # 4 input chunks
CB = HC * BS          # 256 packed columns: col = chunk*BS + b

BF16 = ml_dtypes.bfloat16


def build_program(t_steps=T):
    import concourse.mybir as mybir
    import concourse.tile as tile
    from concourse import bacc
    from concourse.masks import make_identity
    from concourse.tile import add_dep_helper

    f32 = mybir.dt.float32
    bf16 = mybir.dt.bfloat16
    AF = mybir.ActivationFunctionType
    TB = t_steps * BS
    H2 = CB // 2          # 128 packed cols per half
    KH = HC // 2          # 4 chunks per half

    nc = bacc.Bacc("TRN2", target_bir_lowering=False)

    # ---- DRAM I/O (per core; weights replicated by the host) ----
    xt = nc.dram_tensor("xt", [KC, 128, TB], bf16, kind="ExternalInput")
    tdt = nc.dram_tensor("tdt", [KC, 128, TB], bf16, kind="ExternalInput")
    w1h = nc.dram_tensor("w1h", [128, HC, H], bf16, kind="ExternalInput")
    w2h = nc.dram_tensor("w2h", [128, HC, H], bf16, kind="ExternalInput")
    w3h = nc.dram_tensor("w3h", [128, HC, H], bf16, kind="ExternalInput")
    w1x = nc.dram_tensor("w1x", [128, KC, H], bf16, kind="ExternalInput")
    w2x = nc.dram_tensor("w2x", [128, KC, H], bf16, kind="ExternalInput")
    w3x = nc.dram_tensor("w3x", [128, KC, H], bf16, kind="ExternalInput")
    wbt = nc.dram_tensor("wbt", [128, KC, H], bf16, kind="ExternalInput")
    b1t = nc.dram_tensor("b1t", [128, HC], f32, kind="ExternalInput")
    b2t = nc.dram_tensor("b2t", [128, HC], f32, kind="ExternalInput")
    b3t = nc.dram_tensor("b3t", [128, HC], f32, kind="ExternalInput")
    bbt = nc.dram_tensor("bbt", [128, HC], f32, kind="ExternalInput")
    wot = nc.dram_tensor("wot", [128, HC], f32, kind="ExternalInput")
    bot = nc.dram_tensor("bot", [1, 1], f32, kind="ExternalInput")
    out = nc.dram_tensor("out", [BS, 1], f32, kind="ExternalOutput")

    # internal scratch: precomputed per-step gate inputs, t-major, p-outer:
    # pre_*[p, t, c*BS+b].  belta kept fp32 (feeds fp32 decay muls); the
    # gate x-contributions are only ever matmul rhs -> bf16.
    pre_b = nc.dram_tensor("pre_b", [128, t_steps, CB], f32)
    pre_u = nc.dram_tensor("pre_u", [128, t_steps, CB], bf16)
    pre_r = nc.dram_tensor("pre_r", [128, t_steps, CB], bf16)
    pre_n = nc.dram_tensor("pre_n", [128, t_steps, CB], bf16)

    with tile.TileContext(nc) as tc:
        with tc.tile_pool(name="singles", bufs=1) as singles:
            # persistent SBUF: recurrent weights, head, identity, state
            sb_w1h = singles.tile([128, HC, H], bf16)
            sb_w2h = singles.tile([128, HC, H], bf16)
            sb_w3h = singles.tile([128, HC, H], bf16)
            sb_wo = singles.tile([128, HC], f32)
            sb_bo = singles.tile([1, 1], f32)
            ident = singles.tile([128, 128], bf16)
            make_identity(nc, ident)
            st_lo = singles.tile([128, H2], f32)
            st_hi = singles.tile([128, H2], f32)
            nc.vector.memset(st_lo, 0.0)
            nc.vector.memset(st_hi, 0.0)

            # ---- phase 1: precompute belta / xu / xr / xn ----
            with (
                tc.tile_pool(name="prew", bufs=1) as prew,
                tc.tile_pool(name="pspre", bufs=6, space="PSUM") as pspre,
                tc.tile_pool(name="blkp", bufs=3) as blkp,
                tc.tile_pool(name="tmpp", bufs=3) as tmpp,
            ):
                # per-chunk input tiles so the first matmuls only wait on
                # their own chunk's DMA
                sb_tdt = [prew.tile([128, TB], bf16, name=f"sb_tdt{k}") for k in range(KC)]
                sb_wbt = prew.tile([128, KC, H], bf16)
                sb_bb = prew.tile([128, HC], f32)
                nc.sync.dma_start(out=sb_bb, in_=bbt[:, :])
                nc.sync.dma_start(out=sb_wbt, in_=wbt[:, :, :])
                for k in range(KC):
                    nc.sync.dma_start(out=sb_tdt[k], in_=tdt[k, :, :])
                sb_xt = [prew.tile([128, TB], bf16, name=f"sb_xt{k}") for k in range(KC)]
                for k in range(KC):
                    nc.sync.dma_start(out=sb_xt[k], in_=xt[k, :, :])
                sb_w1x = prew.tile([128, KC, H], bf16)
                sb_w2x = prew.tile([128, KC, H], bf16)
                sb_w3x = prew.tile([128, KC, H], bf16)
                nc.sync.dma_start(out=sb_w1x, in_=w1x[:, :, :])
                nc.sync.dma_start(out=sb_w2x, in_=w2x[:, :, :])
                nc.sync.dma_start(out=sb_w3x, in_=w3x[:, :, :])
                sb_b1 = prew.tile([128, HC], f32)
                sb_b2 = prew.tile([128, HC], f32)
                sb_b3 = prew.tile([128, HC], f32)
                nc.sync.dma_start(out=sb_b1, in_=b1t[:, :])
                nc.sync.dma_start(out=sb_b2, in_=b2t[:, :])
                nc.sync.dma_start(out=sb_b3, in_=b3t[:, :])

                SC = min(512, TB)       # psum cols per tile
                TS = SC // BS           # t-steps per block
                NS = TB // SC           # blocks
                jobs = [
                    ("b", sb_wbt, sb_tdt, sb_bb, pre_b, f32, True),
                    ("u", sb_w1x, sb_xt, sb_b1, pre_u, bf16, False),
                    ("r", sb_w2x, sb_xt, sb_b2, pre_r, bf16, False),
                    ("n", sb_w3x, sb_xt, sb_b3, pre_n, bf16, False),
                ]
                for jobi, (nm, wsb, rsb, bsb, dst, odt, is_belta) in enumerate(jobs):
                    if jobi == 1:
                        # recurrent weights transfer while jobs 1-3 compute
                        nc.sync.dma_start(out=sb_w1h, in_=w1h[:, :, :])
                        nc.sync.dma_start(out=sb_w2h, in_=w2h[:, :, :])
                        nc.sync.dma_start(out=sb_w3h, in_=w3h[:, :, :])
                        nc.sync.dma_start(out=sb_wo, in_=wot[:, :])
                        nc.sync.dma_start(out=sb_bo, in_=bot[:, :])
                    for s in range(NS):
                        blk = blkp.tile([128, TS, CB], odt, tag="blk")
                        for m in range(HC):
                            ps = pspre.tile([128, SC], f32, tag="ps")
                            for k in range(KC):
                                nc.tensor.matmul(
                                    ps,
                                    wsb[:, k, m * 128 : (m + 1) * 128],
                                    rsb[k][:, s * SC : (s + 1) * SC],
                                    start=(k == 0),
                                    stop=(k == KC - 1),
                                )
                            # out view: [128, TS, BS] slice of blk at chunk m
                            oap = blk[:, :, m * BS : (m + 1) * BS]
                            ps3 = ps.rearrange("p (t b) -> p t b", b=BS)
                            if is_belta:
                                tmp = tmpp.tile([128, SC], f32, tag="tmp")
                                nc.scalar.activation(
                                    tmp, ps, AF.Relu, bias=bsb[:, m : m + 1], scale=1.0
                                )
                                t3 = tmp.rearrange("p (t b) -> p t b", b=BS)
                                nc.scalar.activation(oap, t3, AF.Exp, scale=-1.0)
                            else:
                                nc.vector.tensor_scalar_add(oap, ps3, bsb[:, m : m + 1])
                        nc.sync.dma_start(
                            out=dst[:, s * TS : (s + 1) * TS, :], in_=blk
                        )
                # sb_wbt doubles as the rhs list for belta? no: wsb is the
                # weight tile; rsb for belta is the sb_tdt list (indexed [k]).

            # ---- phase 2: recurrence ----
            with (
                tc.tile_pool(name="ldp", bufs=3) as ldp,
                tc.tile_pool(name="scp", bufs=2) as scp,
                tc.tile_pool(name="psrec", bufs=2, space="PSUM") as psrec,
            ):
                # software-pipelined: step t's tail computes step t+1's
                # decayed-state tiles right after each half of the state
                # update.  Gate phases are ordered r, u-lo, n-lo, u-hi, n-hi
                # with k-outer matmul loops so the next step's first matmuls
                # only need the low half of the new state.
                sbl_cur = scp.tile([128, H2], bf16, tag="sbl")
                sbh_cur = scp.tile([128, H2], bf16, tag="sbh")
                stm_cur = scp.tile([128, CB], f32, tag="stm")
                nc.vector.memset(sbl_cur, 0.0)
                nc.vector.memset(sbh_cur, 0.0)
                nc.vector.memset(stm_cur, 0.0)
                for t in range(t_steps):
                    ldu = ldp.tile([128, CB], bf16, tag="ldu")
                    ldr = ldp.tile([128, CB], bf16, tag="ldr")
                    ldn = ldp.tile([128, CB], bf16, tag="ldn")
                    nc.sync.dma_start(out=ldu, in_=pre_u[:, t, :])
                    nc.sync.dma_start(out=ldr, in_=pre_r[:, t, :])
                    nc.sync.dma_start(out=ldn, in_=pre_n[:, t, :])
                    if t < t_steps - 1:
                        ldb = ldp.tile([128, CB], f32, tag="ldb")
                        nc.sync.dma_start(out=ldb, in_=pre_b[:, t + 1, :])
                    sbl3 = sbl_cur.rearrange("p (c b) -> p c b", b=BS)
                    sbh3 = sbh_cur.rearrange("p (c b) -> p c b", b=BS)

                    def rhs_chunk(k, lo3=sbl3, hi3=sbh3):
                        return lo3[:, k, :] if k < KH else hi3[:, k - KH, :]

                    # PSUM tiles; x-contributions folded in via identity
                    # matmuls (start flags) so ACT can read PSUM directly
                    psr = psrec.tile([128, CB], f32, tag="psr", bufs=2)
                    psu_lo = psrec.tile([128, H2], f32, tag="psu_lo", bufs=2)
                    psu_hi = psrec.tile([128, H2], f32, tag="psu_hi", bufs=2)
                    psn_lo = psrec.tile([128, H2], f32, tag="psn_lo", bufs=1)
                    psn_hi = psrec.tile([128, H2], f32, tag="psn_hi", bufs=1)
                    ldr3 = ldr.rearrange("p (c b) -> p c b", b=BS)
                    ldu3 = ldu.rearrange("p (c b) -> p c b", b=BS)
                    ldn3 = ldn.rearrange("p (c b) -> p c b", b=BS)
                    for m in range(HC):
                        nc.tensor.matmul(
                            psr[:, m * BS : (m + 1) * BS], ident, ldr3[:, m, :],
                            start=(m == 0), stop=False,
                        )
                    for m in range(HC):
                        pd = psu_lo if m < KH else psu_hi
                        mm = m % KH
                        nc.tensor.matmul(
                            pd[:, mm * BS : (mm + 1) * BS], ident, ldu3[:, m, :],
                            start=(mm == 0), stop=False,
                        )
                    for m in range(HC):
                        pd = psn_lo if m < KH else psn_hi
                        mm = m % KH
                        nc.tensor.matmul(
                            pd[:, mm * BS : (mm + 1) * BS], ident, ldn3[:, m, :],
                            start=(mm == 0), stop=False,
                        )

                    # r gate (k-outer: k<KH rows only need sbl)
                    for k in range(HC):
                        for m in range(HC):
                            nc.tensor.matmul(
                                psr[:, m * BS : (m + 1) * BS],
                                sb_w2h[:, k, m * 128 : (m + 1) * 128],
                                rhs_chunk(k), start=False,
                                stop=(k == HC - 1 and m == HC - 1),
                            )
                    rg = scp.tile([128, CB], f32, tag="rg")
                    nc.scalar.activation(rg, psr, AF.Sigmoid)
                    rs = scp.tile([128, CB], bf16, tag="rs")
                    nc.vector.tensor_mul(rs[:, :H2], rg[:, :H2], stm_cur[:, :H2])
                    nc.vector.tensor_mul(rs[:, H2:], rg[:, H2:], stm_cur[:, H2:])
                    rs3 = rs.rearrange("p (c b) -> p c b", b=BS)

                    # u gate low half (independent of rs -> hides the rs chain)
                    for k in range(HC):
                        for m in range(KH):
                            nc.tensor.matmul(
                                psu_lo[:, m * BS : (m + 1) * BS],
                                sb_w1h[:, k, m * 128 : (m + 1) * 128],
                                rhs_chunk(k), start=False,
                                stop=(k == HC - 1 and m == KH - 1),
                            )
                    ug_lo = scp.tile([128, H2], f32, tag="ug_lo")
                    nc.scalar.activation(ug_lo, psu_lo, AF.Sigmoid)
                    w_lo = scp.tile([128, H2], f32, tag="w_lo")
                    nc.vector.tensor_mul(w_lo, ug_lo, stm_cur[:, :H2])
                    nc.vector.tensor_sub(w_lo, stm_cur[:, :H2], w_lo)

                    # n gate low half
                    for k in range(HC):
                        for m in range(KH):
                            nc.tensor.matmul(
                                psn_lo[:, m * BS : (m + 1) * BS],
                                sb_w3h[:, k, m * 128 : (m + 1) * 128],
                                rs3[:, k, :], start=False,
                                stop=(k == HC - 1 and m == KH - 1),
                            )

                    # u gate high half
                    for k in range(HC):
                        for m in range(KH, HC):
                            nc.tensor.matmul(
                                psu_hi[:, (m - KH) * BS : (m - KH + 1) * BS],
                                sb_w1h[:, k, m * 128 : (m + 1) * 128],
                                rhs_chunk(k), start=False,
                                stop=(k == HC - 1 and m == HC - 1),
                            )
                    ug_hi = scp.tile([128, H2], f32, tag="ug_hi")
                    nc.scalar.activation(ug_hi, psu_hi, AF.Sigmoid)
                    w_hi = scp.tile([128, H2], f32, tag="w_hi")
                    nc.vector.tensor_mul(w_hi, ug_hi, stm_cur[:, H2:])
                    nc.vector.tensor_sub(w_hi, stm_cur[:, H2:], w_hi)

                    # low-half tail while the n-hi matmuls run
                    last = t == t_steps - 1
                    ng_lo = scp.tile([128, H2], f32, tag="ng_lo")
                    nc.scalar.activation(ng_lo, psn_lo, AF.Tanh)
                    e_lo = scp.tile([128, H2], f32, tag="e_lo")
                    nc.vector.tensor_mul(e_lo, ug_lo, ng_lo)
                    nc.vector.tensor_add(st_lo, w_lo, e_lo)
                    lo_tail_inst = None
                    if not last:
                        sbl_cur = scp.tile([128, H2], bf16, tag="sbl")
                        lo_tail_inst = nc.vector.tensor_mul(
                            sbl_cur, st_lo, ldb[:, :H2]
                        )

                    # n gate high half
                    for k in range(HC):
                        for m in range(KH, HC):
                            nc.tensor.matmul(
                                psn_hi[:, (m - KH) * BS : (m - KH + 1) * BS],
                                sb_w3h[:, k, m * 128 : (m + 1) * 128],
                                rs3[:, k, :], start=False,
                                stop=(k == HC - 1 and m == HC - 1),
                            )
                    ng_hi = scp.tile([128, H2], f32, tag="ng_hi")
                    nc.scalar.activation(ng_hi, psn_hi, AF.Tanh)
                    e_hi = scp.tile([128, H2], f32, tag="e_hi")
                    e_hi_inst = nc.vector.tensor_mul(e_hi, ug_hi, ng_hi)
                    if lo_tail_inst is not None:
                        add_dep_helper(
                            e_hi_inst.ins, lo_tail_inst.ins, sync=False,
                            reason="lo tail chain feeds next-step matmuls",
                        )
                    nc.vector.tensor_add(st_hi, w_hi, e_hi)
                    if not last:
                        sbh_cur = scp.tile([128, H2], bf16, tag="sbh")
                        nc.vector.tensor_mul(sbh_cur, st_hi, ldb[:, H2:])
                        stm_cur = scp.tile([128, CB], f32, tag="stm")
                        nc.vector.tensor_mul(stm_cur[:, :H2], st_lo, ldb[:, :H2])
                        nc.vector.tensor_mul(stm_cur[:, H2:], st_hi, ldb[:, H2:])

                # ---- head: out = sigmoid(s @ Wo^T + bo) ----
                pso = psrec.tile([1, BS], f32, tag="psr", bufs=2)
                stl3 = st_lo.rearrange("p (c b) -> p c b", b=BS)
                sth3 = st_hi.rearrange("p (c b) -> p c b", b=BS)
                for k in range(HC):
                    src = stl3[:, k, :] if k < KH else sth3[:, k - KH, :]
                    nc.tensor.matmul(
                        pso, sb_wo[:, k : k + 1], src,
                        start=(k == 0), stop=(k == HC - 1),
                    )
                ob = scp.tile([1, BS], f32, tag="ob")
                nc.scalar.activation(ob, pso, AF.Sigmoid, bias=sb_bo[0:1, 0:1])
                nc.sync.dma_start(out=out[:, :], in_=ob)

    nc.finalize()
    return nc


def _pack_wh(w):  # [H, H] -> [128, HC, H];  out[p, k, m*128+j] = w[m*128+j, k*128+p]
    return np.ascontiguousarray(
        w.reshape(HC, 128, HC, 128).transpose(3, 2, 0, 1).reshape(128, HC, H)
    ).astype(BF16)


def _pack_wx(w):  # [H, IN] -> [128, KC, H]
    return np.ascontiguousarray(
        w.reshape(HC, 128, KC, 128).transpose(3, 2, 0, 1).reshape(128, KC, H)
    ).astype(BF16)


def _pack_bias(b):  # [H] -> [128, HC]
    return np.ascontiguousarray(b.reshape(HC, 128).T).astype(np.float32)


def _pack_x(xs, t_steps):  # [BS, t, IN] -> [KC, 128, t*BS]
    return np.ascontiguousarray(
        xs.reshape(BS, t_steps, KC, 128).transpose(2, 3, 1, 0).reshape(KC, 128, -1)
    ).astype(BF16)


def prepare_in_maps(x, time_delta, Wb, bb, W1, b1, W2, b2, W3, b3, Wo, bo,
                    t_steps=T, ncores=NCORES):
    x = np.asarray(x, np.float32)
    time_delta = np.asarray(time_delta, np.float32)
    common = {
        "w1h": _pack_wh(np.asarray(W1, np.float32)[:, :H]),
        "w2h": _pack_wh(np.asarray(W2, np.float32)[:, :H]),
        "w3h": _pack_wh(np.asarray(W3, np.float32)[:, :H]),
        "w1x": _pack_wx(np.asarray(W1, np.float32)[:, H:]),
        "w2x": _pack_wx(np.asarray(W2, np.float32)[:, H:]),
        "w3x": _pack_wx(np.asarray(W3, np.float32)[:, H:]),
        "wbt": _pack_wx(np.asarray(Wb, np.float32)),
        "b1t": _pack_bias(np.asarray(b1, np.float32)),
        "b2t": _pack_bias(np.asarray(b2, np.float32)),
        "b3t": _pack_bias(np.asarray(b3, np.float32)),
        "bbt": _pack_bias(np.asarray(bb, np.float32)),
        "wot": _pack_bias(np.asarray(Wo, np.float32).reshape(H)),
        "bot": np.asarray(bo, np.float32).reshape(1, 1),
    }
    in_maps = []
    for i in range(ncores):
        sl = slice(i * BS, (i + 1) * BS)
        m = dict(common)
        m["xt"] = _pack_x(x[sl], t_steps)
        m["tdt"] = _pack_x(time_delta[sl], t_steps)
        in_maps.append(m)
    return in_maps


def run(inputs, trace=False, trace_kwargs=None):
    from concourse.bass_utils import run_bass_kernel_spmd

    nc = build_program()
    in_maps = prepare_in_maps(**inputs)
    res = run_bass_kernel_spmd(
        nc, in_maps, list(range(NCORES)), trace=trace,
        trace_kwargs=trace_kwargs or {},
    )
    outs = np.concatenate(
        [np.asarray(res.results[i]["out"]) for i in range(NCORES)], axis=0
    ).astype(np.float32)
    return outs, res


def kernel(**inputs):
    outs, _ = run(inputs, trace=False)
    return outs

